# revision 1
# baseline (speedup 1.0000x reference)
"""Direction-sharded fp8/DoubleRow Bass kernel for nn_DeepLSTM (8 cores).

Sharding: core c -> (batch group c//2 of 512 rows, LSTM direction c%2 for
layer 0, the OPPOSITE direction for layer 1).  All matmuls fp8(e4m3)+
DoubleRow with N=512 moving columns.  Scales: weights x64, one-hot/h
operands x16 -> PSUM scale 1024, descaled for free in the ACT sigmoid/tanh
bias stage.  The layer0->layer1 exchange between the two direction-cores of
a batch group is a pairwise fp8 AllReduce(sum) of the step-indexed layer0
outputs, split into 3 ascending step-slot parts emitted inside the layer-0
loop.  Because layer 1 runs the opposite direction per core, exchange slots
are consumed in the SAME ascending order the parts complete, hiding the
collective almost entirely; the partner half is recovered uniformly as
sum[s] - own[s] on the VectorEngine, keeping the NEFF SPMD-identical across
cores.  All direction dependence (time order, emb weights, fc half) lives
in per-core input data.
"""

import sys

if "/opt/trn_rl_repo" not in sys.path:
    sys.path.insert(0, "/opt/trn_rl_repo")

import numpy as np
import ml_dtypes

B, MAXLEN, H, T, NCLS = 2048, 1000, 512, 21, 10
NCORES = 8
BL = 512                 # batch rows per core (4 groups x 2 dirs)
P = 128
MPAD = 1024
KIN = MPAD // P          # 8
KH = H // P              # 4
KCH = KIN + KH           # 12
G4 = 4 * H
GCH = G4 // P            # 16
WSCALE = 64.0            # fp8 weight scale
HSCALE = 16.0            # fp8 activation / one-hot scale
DESCALE = 1.0 / (WSCALE * HSCALE)

bf16 = ml_dtypes.bfloat16

_NC = None


def _gate_perm():
    idx = np.arange(G4)
    return np.concatenate([idx[0:H], idx[H:2 * H], idx[3 * H:4 * H], idx[2 * H:3 * H]])


def _prep_dir(w_ih, w_hh, b_ih, b_hh, fp8np, in_row_order=None):
    """-> (wT [P,KCH,4H] fp8 scaled, bias [P,GCH] f32). in_row_order optionally
    permutes the input rows (used to put [own-half, other-half] first for L1)."""
    kin = w_ih.shape[1]
    perm = _gate_perm()
    wi = np.asarray(w_ih, np.float32).T            # [kin, 4H]
    if in_row_order is not None:
        wi = wi[in_row_order]
    wt = np.zeros((MPAD + H, G4), np.float32)
    wt[:kin, :] = wi
    wt[MPAD:MPAD + H, :] = np.asarray(w_hh, np.float32).T
    wt = wt[:, perm] * WSCALE
    wt = wt.reshape(KCH, P, G4).transpose(1, 0, 2)
    be = (np.asarray(b_ih, np.float32) + np.asarray(b_hh, np.float32))[perm]
    be = be.reshape(GCH, P).T
    return np.ascontiguousarray(wt.astype(fp8np)), np.ascontiguousarray(
        be.astype(np.float32))


def _prepare_inputs(inputs):
    import concourse.mybir as mybir
    fp8np = mybir.dt.np(mybir.dt.float8e4)

    x = np.asarray(inputs["x"])
    emb_w = np.asarray(inputs["emb_w"], np.float32).reshape(-1)
    emb_b = np.asarray(inputs["emb_b"], np.float32).reshape(-1)[0]
    fc_w = np.asarray(inputs["fc_w"], np.float32)
    fc_b = np.asarray(inputs["fc_b"], np.float32)

    in_maps = []
    fcb_eff = fc_b + emb_b * fc_w.sum(axis=1)
    for c in range(NCORES):
        ib, d = c // 2, c % 2
        m = {}
        # layer0 weights for this direction
        wt0, b0 = _prep_dir(np.asarray(inputs["w_ih0"])[d],
                            np.asarray(inputs["w_hh0"])[d],
                            np.asarray(inputs["b_ih0"])[d],
                            np.asarray(inputs["b_hh0"])[d], fp8np)
        # layer1 runs the OPPOSITE direction on this core (d1 = 1-d), so
        # exchange slots are consumed in the same ascending order they are
        # produced; input rows reordered to [local-half(h_d), partner-half]
        d1 = 1 - d
        wiT = np.asarray(inputs["w_ih1"])[d1].astype(np.float32).T  # [2H, 4H]
        wi1 = np.concatenate([wiT[d * H:(d + 1) * H],
                              wiT[(1 - d) * H:(2 - d) * H]], axis=0)
        wt1, b1 = _prep_dir(wi1.T,
                            np.asarray(inputs["w_hh1"])[d1],
                            np.asarray(inputs["b_ih1"])[d1],
                            np.asarray(inputs["b_hh1"])[d1], fp8np)
        m["wt0"], m["bias0"] = wt0, b0
        m["wt1"], m["bias1"] = wt1, b1
        # x shard (batch group), transposed+padded, bf16
        xs = np.full((MPAD, BL), 255.0, np.float32)
        xs[:MAXLEN, :] = x[ib * BL:(ib + 1) * BL, :].T.astype(np.float32)
        m["xT"] = np.ascontiguousarray(
            xs.reshape(KIN, P, BL).transpose(1, 0, 2).astype(bf16))
        # per-step time index (t = s for fwd, 20-s for bwd), broadcast to 128
        tid = np.arange(T) if d == 0 else (T - 1 - np.arange(T))
        m["tvec"] = np.ascontiguousarray(
            np.broadcast_to(tid[None, :], (P, T)).astype(np.float32))
        # per-step emb weight / fc half follow layer1's direction (d1)
        tid1 = tid[::-1]
        m["embw"] = np.ascontiguousarray(
            np.broadcast_to(emb_w[tid1][None, :], (P, T)).astype(np.float32))
        fch = fc_w[:, d1 * H:(d1 + 1) * H].T
        m["fcT"] = np.ascontiguousarray(
            fch.reshape(KH, P, NCLS).transpose(1, 0, 2).astype(np.float32))
        in_maps.append(m)
    return in_maps, fcb_eff


def _build():
    from contextlib import ExitStack
    import concourse.bacc as bacc
    import concourse.tile as tile
    import concourse.mybir as mybir

    f32 = mybir.dt.float32
    bft = mybir.dt.bfloat16
    fp8 = mybir.dt.float8e4
    SIG = mybir.ActivationFunctionType.Sigmoid
    TANH = mybir.ActivationFunctionType.Tanh
    MUL = mybir.AluOpType.mult
    ADD = mybir.AluOpType.add
    SUB = mybir.AluOpType.subtract
    EQ = mybir.AluOpType.is_equal
    DR = mybir.MatmulPerfMode.DoubleRow

    nc = bacc.Bacc("TRN2", target_bir_lowering=False, debug=False,
                   num_devices=NCORES)

    wt_d = {l: nc.dram_tensor(f"wt{l}", [P, KCH, G4], fp8,
                              kind="ExternalInput").ap() for l in (0, 1)}
    bias_d = {l: nc.dram_tensor(f"bias{l}", [P, GCH], f32,
                                kind="ExternalInput").ap() for l in (0, 1)}
    xT_d = nc.dram_tensor("xT", [P, KIN, BL], bft, kind="ExternalInput").ap()
    tvec_d = nc.dram_tensor("tvec", [P, T], f32, kind="ExternalInput").ap()
    embw_d = nc.dram_tensor("embw", [P, T], f32, kind="ExternalInput").ap()
    fcT_d = nc.dram_tensor("fcT", [P, KH, NCLS], f32, kind="ExternalInput").ap()
    out_d = nc.dram_tensor("out", [NCLS, BL], f32, kind="ExternalOutput").ap()

    with tile.TileContext(nc) as tc, ExitStack() as ctx:
        wpool = ctx.enter_context(tc.tile_pool(name="w", bufs=1))
        cpool = ctx.enter_context(tc.tile_pool(name="const", bufs=1))
        gpool = ctx.enter_context(tc.tile_pool(name="gates", bufs=2))
        spool = ctx.enter_context(tc.tile_pool(name="state", bufs=2))
        tpool = ctx.enter_context(tc.tile_pool(name="tmp", bufs=1))
        iopool = ctx.enter_context(tc.tile_pool(name="io", bufs=3))
        psum = ctx.enter_context(tc.tile_pool(name="psum", bufs=1, space="PSUM"))
        dram = ctx.enter_context(tc.tile_pool(name="dram", bufs=1, space="DRAM"))

        bias_sb = {}
        for l in (0, 1):
            bsb = cpool.tile([P, GCH], f32, name=f"bias{l}")
            nc.sync.dma_start(bsb[:], bias_d[l][:])
            bias_sb[l] = bsb
        tvec_sb = cpool.tile([P, T], f32, name="tvec")
        nc.sync.dma_start(tvec_sb[:], tvec_d[:])
        embw_sb = cpool.tile([P, T], f32, name="embw")
        nc.sync.dma_start(embw_sb[:], embw_d[:])
        fcT_sb = cpool.tile([P, KH, NCLS], f32, name="fcT")
        nc.sync.dma_start(fcT_sb[:], fcT_d[:])
        xT_sb = cpool.tile([P, KIN, BL], bft, name="xT")
        for k in range(KIN):
            nc.sync.dma_start(xT_sb[:, k, :], xT_d[:, k, :])

        # step-indexed layer0 outputs (fp8, x16): own rhs + exchange input
        seq8 = dram.tile([T, P, KH, BL], fp8, name="seq8")
        seqsum = dram.tile([T, P, KH, BL], fp8, name="seqsum")

        z_sb = None
        for layer in (0, 1):
            w_sb = wpool.tile([P, KCH, G4], fp8, name=f"w{layer}", tag=f"w{layer}")
            for k in range(KCH):
                nc.sync.dma_start(w_sb[:, k, :], wt_d[layer][:, k, :])
            if layer == 1:
                z_sb = iopool.tile([P, KH, BL], f32, name="z", tag="z", bufs=1)

            h_prev = None
            c_prev = None
            for s in range(T):
                # ---- input-part moving operand (fp8, scaled x16)
                rhs_in = iopool.tile([P, KIN, BL], fp8, name="rhs", tag="rhs")
                if layer == 0:
                    for k in range(KIN):
                        nc.gpsimd.tensor_scalar(
                            rhs_in[:, k, :], xT_sb[:, k, :],
                            tvec_sb[:, s:s + 1], HSCALE, EQ, MUL)
                else:
                    # layer1 runs the opposite direction: local half at
                    # slot T-1-s; partner half = sum[s] - own[s] (ascending,
                    # matching the order the collective parts complete)
                    nc.sync.dma_start(rhs_in[:, 0:KH, :], seq8[T - 1 - s])
                    osum = iopool.tile([P, KH, BL], fp8, name="osum", tag="osum")
                    oown = iopool.tile([P, KH, BL], fp8, name="oown", tag="oown")
                    nc.sync.dma_start(osum[:], seqsum[s])
                    nc.sync.dma_start(oown[:], seq8[s])
                    nc.vector.tensor_tensor(rhs_in[:, KH:KIN, :], osum[:],
                                            oown[:], SUB)

                nkp = KIN // 2 if s == 0 else KCH // 2  # DoubleRow k-pairs
                # ---- gate matmuls: 16 g-chunks x nkp DoubleRow pairs.
                # waves chosen so c-update can start early: A={i,g}, B={f,o}
                waveA = [0, 1, 2, 3, 12, 13, 14, 15]
                waveB = [4, 5, 6, 7, 8, 9, 10, 11]
                bank = {0: 0, 1: 1, 2: 2, 3: 3, 12: 4, 13: 5, 14: 6, 15: 7,
                        4: 0, 5: 1, 6: 2, 7: 3, 8: 4, 9: 5, 10: 6, 11: 7}
                ps = {}

                def mm_chunk(gc, kps):
                    for kp in kps:
                        if kp < KIN // 2:
                            rhs = rhs_in[:, 2 * kp:2 * kp + 2, :]
                        else:
                            j = 2 * (kp - KIN // 2)
                            rhs = h_prev[:, j:j + 2, :]
                        nc.tensor.matmul(
                            ps[gc][:],
                            w_sb[:, 2 * kp:2 * kp + 2, gc * P:(gc + 1) * P],
                            rhs, start=(kp == 0), stop=(kp == nkp - 1),
                            perf_mode=DR)

                gi = gpool.tile([P, KH, BL], bft, name="gi", tag="gi")
                gf = gpool.tile([P, KH, BL], f32, name="gf", tag="gf")
                go = gpool.tile([P, KH, BL], f32, name="go", tag="go")
                gg = gpool.tile([P, KH, BL], bft, name="gg", tag="gg")
                gtiles = (gi, gf, go, gg)

                def act_chunk(gc):
                    kind = gc // 4
                    if s == 0 and kind == 1:
                        return
                    dst = gtiles[kind][:, gc % 4, :]
                    func = TANH if kind == 3 else SIG
                    nc.scalar.activation(dst, ps[gc][:], func,
                                         bias=bias_sb[layer][:, gc:gc + 1],
                                         scale=DESCALE)

                c_new = spool.tile([P, KH, BL], f32, name="c", tag="c")
                tch = tpool.tile([P, KH, BL], f32, name="tch", tag="tch")
                h_new = spool.tile([P, KH, BL], bft, name="h", tag="h")
                h8 = spool.tile([P, KH, BL], fp8, name="h8", tag="h8", bufs=3)
                ig = tpool.tile([P, KH, BL], f32, name="ig", tag="ig")

                # wave A: input kps, then recurrent kps, then ACTs (i/g
                # interleaved so the first ig half unblocks early)
                for gc in waveA:
                    ps[gc] = psum.tile([P, BL], f32, name=f"ps{gc}",
                                       tag=f"ps{bank[gc]}")
                    mm_chunk(gc, range(KIN // 2))
                if s > 0:
                    for gc in waveA:
                        mm_chunk(gc, range(KIN // 2, nkp))
                for gc in (0, 1, 12, 13, 2, 3, 14, 15):
                    act_chunk(gc)
                # ig = sig(i)*tanh(g), by halves (overlaps wave B matmuls)
                nc.vector.tensor_tensor(ig[:, 0:2, :], gi[:, 0:2, :],
                                        gg[:, 0:2, :], MUL)
                nc.vector.tensor_tensor(ig[:, 2:4, :], gi[:, 2:4, :],
                                        gg[:, 2:4, :], MUL)

                # wave B
                for gc in waveB:
                    ps[gc] = psum.tile([P, BL], f32, name=f"ps{gc}",
                                       tag=f"ps{bank[gc]}")
                    mm_chunk(gc, range(KIN // 2))
                if s > 0:
                    for gc in waveB:
                        mm_chunk(gc, range(KIN // 2, nkp))
                for gc in (4, 5, 8, 9, 6, 7, 10, 11):
                    act_chunk(gc)

                # ---- state update, by h-chunk halves (01 first: it feeds
                # the next step's first recurrent kp)
                fct = tpool.tile([P, KH, BL], f32, name="fct", tag="fct")
                for lo, hi in ((0, 2), (2, 4)):
                    sl = slice(lo, hi)
                    if s > 0:
                        nc.vector.tensor_tensor(fct[:, sl, :], gf[:, sl, :],
                                                c_prev[:, sl, :], MUL)
                        nc.vector.tensor_tensor(c_new[:, sl, :], ig[:, sl, :],
                                                fct[:, sl, :], ADD)
                    else:
                        nc.vector.tensor_copy(c_new[:, sl, :], ig[:, sl, :])
                    nc.scalar.activation(tch[:, sl, :], c_new[:, sl, :], TANH)
                    nc.vector.tensor_tensor(h_new[:, sl, :], go[:, sl, :],
                                            tch[:, sl, :], MUL)
                    nc.scalar.activation(h8[:, sl, :], h_new[:, sl, :],
                                         mybir.ActivationFunctionType.Copy,
                                         scale=HSCALE)
                c_prev, h_prev = c_new, h8

                # ---- per-step outputs
                if layer == 0:
                    nc.sync.dma_start(seq8[s], h8[:])
                    # pairwise AllReduce(sum) of completed step-slot groups,
                    # ascending so early parts hide under layer-0 compute;
                    # layer1 step s consumes seqsum[20-s], so only the last
                    # (smallest) parts sit on the critical path.
                    part = {6: (0, 7), 13: (7, 14), 20: (14, 21)}.get(s)
                    if part is not None:
                        lo, hi = part
                        nc.gpsimd.collective_compute(
                            "AllReduce", ADD,
                            replica_groups=[[0, 1], [2, 3], [4, 5], [6, 7]],
                            ins=[seq8[lo:hi]], outs=[seqsum[lo:hi]])
                else:
                    emb_ap = embw_sb[:, s:s + 1]
                    if s == 0:
                        nc.vector.tensor_scalar(z_sb[:], h_new[:], emb_ap,
                                                None, MUL)
                    else:
                        zt = tpool.tile([P, KH, BL], f32, name="zt", tag="zt")
                        nc.vector.tensor_scalar(zt[:], h_new[:], emb_ap,
                                                None, MUL)
                        nc.vector.tensor_tensor(z_sb[:], z_sb[:], zt[:], ADD)

        # final fc partial: out[n, b] = sum_k fcT[k, n] * z[k, b] (dir half)
        ps_fc = psum.tile([P, BL], f32, name="psfc", tag="ps0")
        for j in range(KH):
            nc.tensor.matmul(ps_fc[:NCLS, :], fcT_sb[:, j, :], z_sb[:, j, :],
                             start=(j == 0), stop=(j == KH - 1))
        out_sb = tpool.tile([P, BL], f32, name="outsb", tag="outsb")
        nc.vector.tensor_copy(out_sb[:NCLS, :], ps_fc[:NCLS, :])
        nc.sync.dma_start(out_d[:], out_sb[:NCLS, :])

    nc.finalize()
    return nc


def _get_nc():
    global _NC
    if _NC is None:
        _NC = _build()
    return _NC


def _run(inputs, trace=False, **kw):
    from concourse.bass_utils import run_bass_kernel_spmd
    nc = _get_nc()
    in_maps, fcb_eff = _prepare_inputs(inputs)
    res = run_bass_kernel_spmd(nc, in_maps, core_ids=list(range(NCORES)),
                               trace=trace, **kw)
    out = np.empty((B, NCLS), np.float32)
    for ib in range(NCORES // 2):
        pf = np.asarray(res.results[2 * ib]["out"]).T
        pb = np.asarray(res.results[2 * ib + 1]["out"]).T
        out[ib * BL:(ib + 1) * BL, :] = pf + pb + fcb_eff[None, :]
    return out, res


def kernel(**inputs):
    return _run(inputs, trace=False)[0]



# revision 9
# speedup vs baseline: 637021.0000x; 637021.0000x over previous
"""Direction-sharded fp8/DoubleRow Bass kernel for nn_DeepLSTM (8 cores).

Sharding: core c -> (batch group c//2 of 512 rows, LSTM direction c%2 for
layer 0, the OPPOSITE direction for layer 1).  All matmuls fp8(e4m3)+
DoubleRow with N=512 moving columns.  Scales: weights x64, one-hot/h
operands x16 -> PSUM scale 1024, descaled for free in the ACT sigmoid/tanh
bias stage.  The layer0->layer1 exchange between the two direction-cores of
a batch group is a pairwise fp8 AllReduce(sum) of the step-indexed layer0
outputs, split into 3 ascending step-slot parts emitted inside the layer-0
loop.  Because layer 1 runs the opposite direction per core, exchange slots
are consumed in the SAME ascending order the parts complete, hiding the
collective almost entirely; the partner half is recovered uniformly as
sum[s] - own[s] on the VectorEngine, keeping the NEFF SPMD-identical across
cores.  All direction dependence (time order, emb weights, fc half) lives
in per-core input data.
"""

import sys

if "/opt/trn_rl_repo" not in sys.path:
    sys.path.insert(0, "/opt/trn_rl_repo")

import numpy as np
import ml_dtypes

B, MAXLEN, H, T, NCLS = 2048, 1000, 512, 21, 10
NCORES = 8
BL = 512                 # batch rows per core (4 groups x 2 dirs)
P = 128
MPAD = 1024
KIN = MPAD // P          # 8
KH = H // P              # 4
KCH = KIN + KH           # 12
G4 = 4 * H
GCH = G4 // P            # 16
WSCALE = 64.0            # fp8 weight scale
HSCALE = 16.0            # fp8 activation / one-hot scale
DESCALE = 1.0 / (WSCALE * HSCALE)

bf16 = ml_dtypes.bfloat16

_NC = None


def _gate_perm():
    idx = np.arange(G4)
    return np.concatenate([idx[0:H], idx[H:2 * H], idx[3 * H:4 * H], idx[2 * H:3 * H]])


def _prep_dir(w_ih, w_hh, b_ih, b_hh, fp8np, in_row_order=None):
    """-> (wT [P,KCH,4H] fp8 scaled, bias [P,GCH] f32). in_row_order optionally
    permutes the input rows (used to put [own-half, other-half] first for L1)."""
    kin = w_ih.shape[1]
    perm = _gate_perm()
    wi = np.asarray(w_ih, np.float32).T            # [kin, 4H]
    if in_row_order is not None:
        wi = wi[in_row_order]
    wt = np.zeros((MPAD + H, G4), np.float32)
    wt[:kin, :] = wi
    wt[MPAD:MPAD + H, :] = np.asarray(w_hh, np.float32).T
    wt = wt[:, perm] * WSCALE
    wt = wt.reshape(KCH, P, G4).transpose(1, 0, 2)
    be = (np.asarray(b_ih, np.float32) + np.asarray(b_hh, np.float32))[perm]
    be = be.reshape(GCH, P).T
    return np.ascontiguousarray(wt.astype(fp8np)), np.ascontiguousarray(
        be.astype(np.float32))


def _prepare_inputs(inputs):
    import concourse.mybir as mybir
    fp8np = mybir.dt.np(mybir.dt.float8e4)

    x = np.asarray(inputs["x"])
    emb_w = np.asarray(inputs["emb_w"], np.float32).reshape(-1)
    emb_b = np.asarray(inputs["emb_b"], np.float32).reshape(-1)[0]
    fc_w = np.asarray(inputs["fc_w"], np.float32)
    fc_b = np.asarray(inputs["fc_b"], np.float32)

    in_maps = []
    fcb_eff = fc_b + emb_b * fc_w.sum(axis=1)
    for c in range(NCORES):
        ib, d = c // 2, c % 2
        m = {}
        # layer0 weights for this direction
        wt0, b0 = _prep_dir(np.asarray(inputs["w_ih0"])[d],
                            np.asarray(inputs["w_hh0"])[d],
                            np.asarray(inputs["b_ih0"])[d],
                            np.asarray(inputs["b_hh0"])[d], fp8np)
        # layer1 runs the OPPOSITE direction on this core (d1 = 1-d), so
        # exchange slots are consumed in the same ascending order they are
        # produced; input rows reordered to [local-half(h_d), partner-half]
        d1 = 1 - d
        wiT = np.asarray(inputs["w_ih1"])[d1].astype(np.float32).T  # [2H, 4H]
        wi1 = np.concatenate([wiT[d * H:(d + 1) * H],
                              wiT[(1 - d) * H:(2 - d) * H]], axis=0)
        wt1, b1 = _prep_dir(wi1.T,
                            np.asarray(inputs["w_hh1"])[d1],
                            np.asarray(inputs["b_ih1"])[d1],
                            np.asarray(inputs["b_hh1"])[d1], fp8np)
        m["wt0"], m["bias0"] = wt0, b0
        m["wt1"], m["bias1"] = wt1, b1
        # x shard (batch group), transposed+padded, bf16
        xs = np.full((MPAD, BL), 255.0, np.float32)
        xs[:MAXLEN, :] = x[ib * BL:(ib + 1) * BL, :].T.astype(np.float32)
        m["xT"] = np.ascontiguousarray(
            xs.reshape(KIN, P, BL).transpose(1, 0, 2).astype(bf16))
        # per-step time index (t = s for fwd, 20-s for bwd), broadcast to 128
        tid = np.arange(T) if d == 0 else (T - 1 - np.arange(T))
        m["tvec"] = np.ascontiguousarray(
            np.broadcast_to(tid[None, :], (P, T)).astype(np.float32))
        # per-step emb weight / fc half follow layer1's direction (d1)
        tid1 = tid[::-1]
        m["embw"] = np.ascontiguousarray(
            np.broadcast_to(emb_w[tid1][None, :], (P, T)).astype(np.float32))
        fch = fc_w[:, d1 * H:(d1 + 1) * H].T
        m["fcT"] = np.ascontiguousarray(
            fch.reshape(KH, P, NCLS).transpose(1, 0, 2).astype(np.float32))
        in_maps.append(m)
    return in_maps, fcb_eff


def _build():
    from contextlib import ExitStack
    import concourse.bacc as bacc
    import concourse.tile as tile
    import concourse.mybir as mybir

    f32 = mybir.dt.float32
    bft = mybir.dt.bfloat16
    fp8 = mybir.dt.float8e4
    SIG = mybir.ActivationFunctionType.Sigmoid
    TANH = mybir.ActivationFunctionType.Tanh
    MUL = mybir.AluOpType.mult
    ADD = mybir.AluOpType.add
    SUB = mybir.AluOpType.subtract
    EQ = mybir.AluOpType.is_equal
    DR = mybir.MatmulPerfMode.DoubleRow

    nc = bacc.Bacc("TRN2", target_bir_lowering=False, debug=False,
                   num_devices=NCORES)

    wt_d = {l: nc.dram_tensor(f"wt{l}", [P, KCH, G4], fp8,
                              kind="ExternalInput").ap() for l in (0, 1)}
    bias_d = {l: nc.dram_tensor(f"bias{l}", [P, GCH], f32,
                                kind="ExternalInput").ap() for l in (0, 1)}
    xT_d = nc.dram_tensor("xT", [P, KIN, BL], bft, kind="ExternalInput").ap()
    tvec_d = nc.dram_tensor("tvec", [P, T], f32, kind="ExternalInput").ap()
    embw_d = nc.dram_tensor("embw", [P, T], f32, kind="ExternalInput").ap()
    fcT_d = nc.dram_tensor("fcT", [P, KH, NCLS], f32, kind="ExternalInput").ap()
    out_d = nc.dram_tensor("out", [NCLS, BL], f32, kind="ExternalOutput").ap()

    with tile.TileContext(nc) as tc, ExitStack() as ctx:
        wpool = ctx.enter_context(tc.tile_pool(name="w", bufs=1))
        cpool = ctx.enter_context(tc.tile_pool(name="const", bufs=1))
        gpool = ctx.enter_context(tc.tile_pool(name="gates", bufs=2))
        spool = ctx.enter_context(tc.tile_pool(name="state", bufs=2))
        tpool = ctx.enter_context(tc.tile_pool(name="tmp", bufs=1))
        iopool = ctx.enter_context(tc.tile_pool(name="io", bufs=3))
        psum = ctx.enter_context(tc.tile_pool(name="psum", bufs=1, space="PSUM"))
        dram = ctx.enter_context(tc.tile_pool(name="dram", bufs=1, space="DRAM"))

        bias_sb = {}
        for l in (0, 1):
            bsb = cpool.tile([P, GCH], f32, name=f"bias{l}")
            nc.sync.dma_start(bsb[:], bias_d[l][:])
            bias_sb[l] = bsb
        tvec_sb = cpool.tile([P, T], f32, name="tvec")
        nc.sync.dma_start(tvec_sb[:], tvec_d[:])
        embw_sb = cpool.tile([P, T], f32, name="embw")
        nc.sync.dma_start(embw_sb[:], embw_d[:])
        fcT_sb = cpool.tile([P, KH, NCLS], f32, name="fcT")
        nc.sync.dma_start(fcT_sb[:], fcT_d[:])
        xT_sb = cpool.tile([P, KIN, BL], bft, name="xT")
        for k in range(KIN):
            nc.sync.dma_start(xT_sb[:, k, :], xT_d[:, k, :])

        # step-indexed layer0 outputs (fp8, x16): own rhs + exchange input
        seq8 = dram.tile([T, P, KH, BL], fp8, name="seq8")
        seqsum = dram.tile([T, P, KH, BL], fp8, name="seqsum")

        z_sb = None
        for layer in (0, 1):
            w_sb = wpool.tile([P, KCH, G4], fp8, name=f"w{layer}", tag=f"w{layer}")
            for k in range(KCH):
                nc.sync.dma_start(w_sb[:, k, :], wt_d[layer][:, k, :])
            if layer == 1:
                z_sb = iopool.tile([P, KH, BL], f32, name="z", tag="z", bufs=1)

            h_prev = None
            c_prev = None
            for s in range(T):
                # ---- input-part moving operand (fp8, scaled x16)
                rhs_in = iopool.tile([P, KIN, BL], fp8, name="rhs", tag="rhs",
                                     bufs=8)
                if layer == 0:
                    for k in range(KIN):
                        nc.gpsimd.tensor_scalar(
                            rhs_in[:, k, :], xT_sb[:, k, :],
                            tvec_sb[:, s:s + 1], HSCALE, EQ, MUL)
                else:
                    # layer1 runs the opposite direction: local half at
                    # slot T-1-s; partner half = sum[s] - own[s] (ascending,
                    # matching the order the collective parts complete)
                    nc.sync.dma_start(rhs_in[:, 0:KH, :], seq8[T - 1 - s])
                    osum = iopool.tile([P, KH, BL], fp8, name="osum", tag="osum")
                    oown = iopool.tile([P, KH, BL], fp8, name="oown", tag="oown")
                    nc.sync.dma_start(osum[:], seqsum[s])
                    nc.sync.dma_start(oown[:], seq8[s])
                    nc.vector.tensor_tensor(rhs_in[:, KH:KIN, :], osum[:],
                                            oown[:], SUB)

                nkp = KIN // 2 if s == 0 else KCH // 2  # DoubleRow k-pairs
                # ---- gate matmuls: 16 g-chunks x nkp DoubleRow pairs.
                # waves chosen so c-update can start early: A={i,g}, B={f,o}
                waveA = [0, 1, 2, 3, 12, 13, 14, 15]
                waveB = [4, 5, 6, 7, 8, 9, 10, 11]
                bank = {0: 0, 1: 1, 2: 2, 3: 3, 12: 4, 13: 5, 14: 6, 15: 7,
                        4: 0, 5: 1, 6: 2, 7: 3, 8: 4, 9: 5, 10: 6, 11: 7}
                ps = {}

                def mm_chunk(gc, kps):
                    for kp in kps:
                        if kp < KIN // 2:
                            rhs = rhs_in[:, 2 * kp:2 * kp + 2, :]
                        else:
                            j = 2 * (kp - KIN // 2)
                            rhs = h_prev[:, j:j + 2, :]
                        nc.tensor.matmul(
                            ps[gc][:],
                            w_sb[:, 2 * kp:2 * kp + 2, gc * P:(gc + 1) * P],
                            rhs, start=(kp == 0), stop=(kp == nkp - 1),
                            perf_mode=DR)

                gi = gpool.tile([P, KH, BL], bft, name="gi", tag="gi")
                gf = gpool.tile([P, KH, BL], f32, name="gf", tag="gf")
                go = gpool.tile([P, KH, BL], bft, name="go", tag="go")
                gg = gpool.tile([P, KH, BL], bft, name="gg", tag="gg")
                gtiles = (gi, gf, go, gg)

                def act_chunk(gc):
                    kind = gc // 4
                    if s == 0 and kind == 1:
                        return
                    dst = gtiles[kind][:, gc % 4, :]
                    func = TANH if kind == 3 else SIG
                    nc.scalar.activation(dst, ps[gc][:], func,
                                         bias=bias_sb[layer][:, gc:gc + 1],
                                         scale=DESCALE)

                c_new = spool.tile([P, KH, BL], f32, name="c", tag="c")
                tch = tpool.tile([P, KH, BL], bft, name="tch", tag="tch")
                h_new = spool.tile([P, KH, BL], bft, name="h", tag="h")
                h8 = spool.tile([P, KH, BL], fp8, name="h8", tag="h8", bufs=3)
                ig = tpool.tile([P, KH, BL], bft, name="ig", tag="ig")

                # wave A: input kps, then recurrent kps, then ACTs (i/g
                # interleaved so the first ig half unblocks early)
                for gc in waveA:
                    ps[gc] = psum.tile([P, BL], f32, name=f"ps{gc}",
                                       tag=f"ps{bank[gc]}")
                    mm_chunk(gc, range(KIN // 2))
                if s > 0:
                    for gc in waveA:
                        mm_chunk(gc, range(KIN // 2, nkp))
                for gc in (0, 1, 12, 13, 2, 3, 14, 15):
                    act_chunk(gc)
                # ig = sig(i)*tanh(g), by halves (overlaps wave B matmuls)
                nc.vector.tensor_tensor(ig[:, 0:2, :], gi[:, 0:2, :],
                                        gg[:, 0:2, :], MUL)
                nc.vector.tensor_tensor(ig[:, 2:4, :], gi[:, 2:4, :],
                                        gg[:, 2:4, :], MUL)

                # wave B
                for gc in waveB:
                    ps[gc] = psum.tile([P, BL], f32, name=f"ps{gc}",
                                       tag=f"ps{bank[gc]}")
                    mm_chunk(gc, range(KIN // 2))
                if s > 0:
                    for gc in waveB:
                        mm_chunk(gc, range(KIN // 2, nkp))
                for gc in (4, 5, 8, 9, 6, 7, 10, 11):
                    act_chunk(gc)

                # ---- state update, by h-chunk halves (01 first: it feeds
                # the next step's first recurrent kp)
                fct = tpool.tile([P, KH, BL], f32, name="fct", tag="fct")
                for lo, hi in ((0, 2), (2, 4)):
                    sl = slice(lo, hi)
                    if s > 0:
                        nc.vector.tensor_tensor(fct[:, sl, :], gf[:, sl, :],
                                                c_prev[:, sl, :], MUL)
                        nc.vector.tensor_tensor(c_new[:, sl, :], ig[:, sl, :],
                                                fct[:, sl, :], ADD)
                    else:
                        nc.vector.tensor_copy(c_new[:, sl, :], ig[:, sl, :])
                    nc.scalar.activation(tch[:, sl, :], c_new[:, sl, :], TANH)
                    nc.vector.tensor_tensor(h_new[:, sl, :], go[:, sl, :],
                                            tch[:, sl, :], MUL)
                    # fp8 h8 = 16*h on DVE: off ACT (busiest engine) and off
                    # Pool (whose queue the collectives block)
                    nc.vector.tensor_scalar(h8[:, sl, :], h_new[:, sl, :],
                                            HSCALE, None, MUL)
                c_prev, h_prev = c_new, h8

                # ---- per-step outputs
                if layer == 0:
                    nc.sync.dma_start(seq8[s], h8[:])
                    # pairwise AllReduce(sum) of completed step-slot groups,
                    # ascending so early parts hide under layer-0 compute;
                    # layer1 step s consumes seqsum[20-s], so only the last
                    # (smallest) parts sit on the critical path.
                    part = {6: (0, 7), 13: (7, 14), 20: (14, 21)}.get(s)
                    if part is not None:
                        lo, hi = part
                        nc.gpsimd.collective_compute(
                            "AllReduce", ADD,
                            replica_groups=[[0, 1], [2, 3], [4, 5], [6, 7]],
                            ins=[seq8[lo:hi]], outs=[seqsum[lo:hi]])
                else:
                    emb_ap = embw_sb[:, s:s + 1]
                    if s == 0:
                        nc.vector.tensor_scalar(z_sb[:], h_new[:], emb_ap,
                                                None, MUL)
                    else:
                        zt = tpool.tile([P, KH, BL], f32, name="zt", tag="zt")
                        nc.vector.tensor_scalar(zt[:], h_new[:], emb_ap,
                                                None, MUL)
                        nc.vector.tensor_tensor(z_sb[:], z_sb[:], zt[:], ADD)

        # final fc partial: out[n, b] = sum_k fcT[k, n] * z[k, b] (dir half)
        ps_fc = psum.tile([P, BL], f32, name="psfc", tag="ps0")
        for j in range(KH):
            nc.tensor.matmul(ps_fc[:NCLS, :], fcT_sb[:, j, :], z_sb[:, j, :],
                             start=(j == 0), stop=(j == KH - 1))
        out_sb = tpool.tile([P, BL], f32, name="outsb", tag="outsb")
        nc.vector.tensor_copy(out_sb[:NCLS, :], ps_fc[:NCLS, :])
        nc.sync.dma_start(out_d[:], out_sb[:NCLS, :])

    nc.finalize()
    return nc


def _get_nc():
    global _NC
    if _NC is None:
        _NC = _build()
    return _NC


def _run(inputs, trace=False, **kw):
    from concourse.bass_utils import run_bass_kernel_spmd
    nc = _get_nc()
    in_maps, fcb_eff = _prepare_inputs(inputs)
    res = run_bass_kernel_spmd(nc, in_maps, core_ids=list(range(NCORES)),
                               trace=trace, **kw)
    out = np.empty((B, NCLS), np.float32)
    for ib in range(NCORES // 2):
        pf = np.asarray(res.results[2 * ib]["out"]).T
        pb = np.asarray(res.results[2 * ib + 1]["out"]).T
        out[ib * BL:(ib + 1) * BL, :] = pf + pb + fcb_eff[None, :]
    return out, res


def kernel(**inputs):
    return _run(inputs, trace=False)[0]

